# revision 12
# baseline (speedup 1.0000x reference)
"""CapsuleLayer kernel for 8 Trainium2 NeuronCores.

Math: with b0 = 0, softmax(b0, axis=1) is exactly uniform (1/N), so
outputs[b,i,k] = squash_k((1/N) * sum_j inputs_hat[b,j,k]) independent of i.
The b-update keeps b constant along axis 1, so softmax stays exactly uniform
and all routing iterations return the same outputs. Hence:

    Wsum[m,k] = sum_j W[j,m,k]
    v[b,k]    = (1/N) * (inputs @ Wsum)[b,k]
    out[b,i,k] = squash_k(v)[b,k]          (broadcast over i)

Sharding strategy:
  L1 (m-sharded): core c reduces W[:, 32c:32c+32, :] over j -> Wsum rows.
     W is staged to the device in bf16 (the 2e-2 rel-err budget dwarfs
     bf16's ~2e-3; the problem registry's references are bf16-native).
  L2 (batch-sharded): core c computes s_c = squash((inputs_c @ Wsum)/N)
     [64, 256] — the complete mathematical content of its output shard,
     since the i axis is degenerate.
  Unshard (host): concat s_c over batch and materialize the replicated
     i axis to the full [512, 256, 256] float32 output.
"""

import numpy as np
import ml_dtypes

import concourse.bass as bass
import concourse.mybir as mybir
import concourse.tile as tile
from concourse.ap import AP
from concourse.bass_utils import run_bass_kernel_spmd

F32 = mybir.dt.float32
BF16 = mybir.dt.bfloat16
NP_BF16 = ml_dtypes.bfloat16

B, N = 512, 256
NCORES = 8
BPC = B // NCORES  # 64 batch rows per core (L2)
MPC = N // NCORES  # 32 m rows per core (L1)
EPS = 1e-7

_CACHE = {}


def _fix_multiwait(nc, maxw=1):
    """This walrus build rejects instructions carrying more than one sync
    wait ("Too many sync wait commands"). Hoist extra waits into standalone
    single-wait EventSemaphore instructions on the same engine, placed
    immediately before the offender."""
    ctr = 0
    for fn in nc.m.functions:
        for bb in fn.blocks:
            out = []
            for ins in bb.instructions:
                si = ins.sync_info
                if si is not None and len(si.on_wait) > maxw:
                    waits = list(si.on_wait)
                    for w in waits[:-maxw]:
                        ctr += 1
                        ev = mybir.InstEventSemaphore(
                            name=f"mwsplit-{ctr}",
                            engine=ins.engine,
                            ins=[],
                            outs=[],
                            sync_info=mybir.SyncInfo(on_wait=[w], on_update=[]),
                        )
                        nc.register_instruction(ev, overwrite=True)
                        out.append(ev)
                    si.on_wait = waits[-maxw:]
                    ins.sync_info = si
                out.append(ins)
            bb.instructions[:] = out
    return nc

# Exec times (ns) of the last traced run, for test harnesses.
LAST_EXEC_NS = {"k1": None, "k2": None}


def _build_k1():
    """Reduce the per-core W slice over j (bf16 input, f32 accumulation).

    Input  w_in [128 (j%128), 16384 (jhalf*8192 + m_local*256 + k)] bf16
           (host packs the two j-halves of W[:, mslice, :] side by side)
    Output wsum_part [1, 8192] f32  (= Wsum[mslice, :] flat)

    Per chunk: ONE DMA covering both j-halves (sync queue), DVE adds the
    halves (j 256->128), PE ones-matmuls reduce the 128 partitions into
    PSUM f32, and copies (mostly scalar) drain PSUM into the f32 acc row.
    """
    nc = bass.Bass()
    FREE = MPC * N    # 8192
    MMF = 512         # moving free dim per matmul (one PSUM bank)

    w = nc.dram_tensor("w_in", [128, 2 * FREE], BF16, kind="ExternalInput")
    wsum = nc.dram_tensor("wsum_part", [1, FREE], F32, kind="ExternalOutput")

    # ~1 MB per chunk (both halves) keeps DMA efficiency; the small last
    # chunks shorten the serial tail after the final load.
    CHUNKS = [2048, 2048, 2048, 1024, 512, 512]
    assert sum(CHUNKS) == FREE

    with tile.TileContext(nc) as tc:
        with (
            tc.tile_pool(name="singles", bufs=1) as singles,
            tc.tile_pool(name="psum", bufs=8, space="PSUM") as psum_pool,
        ):
            ones = singles.tile([128, 1], BF16)
            nc.vector.memset(ones[:], 1.0)
            acc = singles.tile([1, FREE], F32)

            # Issue ALL chunk loads first (program order = engine queue
            # order), alternating the two HWDGE queues, so no compute op
            # can delay a load issue.
            tiles = []
            off = 0
            for ci, chunk in enumerate(CHUNKS):
                t2 = singles.tile([128, 2 * chunk], BF16, tag=f"t{ci}")
                src = AP(
                    tensor=w,
                    offset=off,
                    ap=[[2 * FREE, 128], [FREE, 2], [1, chunk]],
                )
                eng = nc.sync if ci % 2 == 0 else nc.scalar
                eng.dma_start(out=t2[:].rearrange(
                    "p (two c) -> p two c", two=2), in_=src)
                tiles.append((off, chunk, t2))
                off += chunk

            gctr = 0
            for ci, (off, chunk, t2) in enumerate(tiles):
                ts = singles.tile([128, chunk], BF16, tag=f"ts{ci}")
                nc.vector.tensor_add(
                    ts[:], t2[:, 0:chunk], t2[:, chunk:2 * chunk]
                )
                for g in range(chunk // MMF):
                    ps = psum_pool.tile([1, MMF], F32)
                    gs = slice(g * MMF, (g + 1) * MMF)
                    nc.tensor.matmul(
                        ps[:], lhsT=ones[:], rhs=ts[:, gs],
                        start=True, stop=True,
                    )
                    osl = slice(off + g * MMF, off + (g + 1) * MMF)
                    # scalar takes most drains; vector gets every 3rd so
                    # the add pipeline keeps flowing.
                    gctr += 1
                    if gctr % 3 == 0:
                        nc.vector.tensor_copy(out=acc[0:1, osl], in_=ps[:])
                    else:
                        nc.scalar.copy(out=acc[0:1, osl], in_=ps[:])

            nc.sync.dma_start(out=wsum[:], in_=acc[:])
    return nc


def _build_k2():
    """Per-core: u = inputs_c @ Wsum, s = squash(u/N); write s [64, 256].

    Inputs  xt   [256 (m), 64 (b)]  bf16 (= inputs_c.T)
            wsum [256 (m), 256 (k)] bf16
    Output  s_out [64, 256] f32 = squash((inputs_c @ Wsum)/N)
    """
    nc = bass.Bass()
    # xw packs inputs_c.T and Wsum column-wise: [256 (m), 64 + 256]
    xw = nc.dram_tensor("xw", [N, BPC + N], BF16, kind="ExternalInput")
    s_out = nc.dram_tensor("s_out", [BPC, N], F32, kind="ExternalOutput")

    with tile.TileContext(nc) as tc:
        with (
            tc.tile_pool(name="sb", bufs=1) as sb,
            tc.tile_pool(name="psum", bufs=1, space="PSUM") as psum_pool,
        ):
            # Contraction dim m on partitions, split into two 128-halves.
            h0 = sb.tile([128, BPC + N], BF16)
            nc.sync.dma_start(out=h0[:], in_=xw[0:128, :])
            h1 = sb.tile([128, BPC + N], BF16)
            nc.scalar.dma_start(out=h1[:], in_=xw[128:256, :])

            # u[b, k] = sum_m inputs_c[b, m] * Wsum[m, k]
            u = psum_pool.tile([BPC, N], F32)
            nc.tensor.matmul(u[:], lhsT=h0[:, 0:BPC], rhs=h0[:, BPC:],
                             start=True, stop=False)
            nc.tensor.matmul(u[:], lhsT=h1[:, 0:BPC], rhs=h1[:, BPC:],
                             start=False, stop=True)

            # squash: v = u/N; s2 = sum_k v^2; s = v * s2/(1+s2)/sqrt(s2+eps)
            #       = u * factor,  factor = s2/(1+s2)/sqrt(s2+eps)/N
            sq = sb.tile([BPC, N], F32)
            s2 = sb.tile([BPC, 1], F32)
            nc.scalar.activation(
                out=sq[:], in_=u[:], func=mybir.ActivationFunctionType.Square,
                scale=1.0 / N, accum_out=s2[:],
            )
            eps_t = sb.tile([BPC, 1], F32)
            nc.vector.memset(eps_t[:], EPS)
            # parallel halves: scalar does r = sqrt(s2+eps) while vector
            # does q = (s2/N) / (1+s2); then s = u * q * (1/r) in one op.
            r = sb.tile([BPC, 1], F32)
            nc.scalar.activation(
                out=r[:], in_=s2[:], func=mybir.ActivationFunctionType.Sqrt,
                bias=eps_t[:],
            )
            r2 = sb.tile([BPC, 1], F32)
            nc.vector.reciprocal(r2[:], r[:])
            den1 = sb.tile([BPC, 1], F32)
            nc.vector.tensor_scalar(
                den1[:], s2[:], 1.0, None, mybir.AluOpType.add
            )
            p2 = sb.tile([BPC, 1], F32)
            nc.vector.reciprocal(p2[:], den1[:])
            q = sb.tile([BPC, 1], F32)
            nc.vector.scalar_tensor_tensor(
                q[:], s2[:], 1.0 / N, p2[:],
                op0=mybir.AluOpType.mult, op1=mybir.AluOpType.mult,
            )

            s_row = sb.tile([BPC, N], F32)
            nc.vector.tensor_scalar(
                s_row[:], u[:], q[:], r2[:],
                mybir.AluOpType.mult, mybir.AluOpType.mult,
            )
            nc.sync.dma_start(out=s_out[:], in_=s_row[:])
    return nc


def _run(nc, in_maps, core_ids, trace):
    if trace:
        try:
            return run_bass_kernel_spmd(nc, in_maps, core_ids, trace=True)
        except Exception as e:  # noqa: BLE001
            print(f"kernel: trace run failed ({e}); rerunning without trace")
    return run_bass_kernel_spmd(nc, in_maps, core_ids, trace=False)


def _get(name):
    if name not in _CACHE:
        _CACHE[name] = _fix_multiwait(_build_k1() if name == "k1" else _build_k2())
    return _CACHE[name]


def kernel(inputs: np.ndarray, W: np.ndarray, trace: bool = False) -> np.ndarray:
    inputs = np.ascontiguousarray(inputs, dtype=np.float32)
    W = np.ascontiguousarray(W, dtype=np.float32)
    core_ids = list(range(NCORES))

    # ---- L1: Wsum rows, m-sharded, bf16 ----
    k1 = _get("k1")
    w_bf = W.astype(NP_BF16)  # host-side staging cast
    # Pack the two j-halves side by side: [128, 2*8192]
    in_maps1 = [
        {
            "w_in": np.ascontiguousarray(
                w_bf[:, c * MPC:(c + 1) * MPC, :]
                .reshape(2, 128, MPC * N)
                .transpose(1, 0, 2)
                .reshape(128, 2 * MPC * N)
            )
        }
        for c in core_ids
    ]
    res1 = _run(k1, in_maps1, core_ids, trace)
    LAST_EXEC_NS["k1"] = res1.exec_time_ns
    wsum = np.concatenate(
        [res1.results[c]["wsum_part"].reshape(MPC, N) for c in core_ids], axis=0
    )  # [256, 256] f32

    # ---- L2: matmul + squash, batch-sharded ----
    k2 = _get("k2")
    xt_full = np.ascontiguousarray(inputs.T).astype(NP_BF16)  # [256, 512]
    wsum_bf = wsum.astype(NP_BF16)
    in_maps2 = [
        {
            "xw": np.ascontiguousarray(np.concatenate(
                [xt_full[:, c * BPC:(c + 1) * BPC], wsum_bf], axis=1
            )),
        }
        for c in core_ids
    ]
    res2 = _run(k2, in_maps2, core_ids, trace)
    LAST_EXEC_NS["k2"] = res2.exec_time_ns

    # ---- unshard: concat batch shards, materialize the replicated i axis ----
    s = np.concatenate(
        [res2.results[c]["s_out"] for c in core_ids], axis=0
    )  # [512, 256] f32
    out = np.ascontiguousarray(
        np.broadcast_to(s[:, None, :], (B, N, N))
    )
    return out


# revision 13
# speedup vs baseline: 1.0234x; 1.0234x over previous
"""CapsuleLayer kernel for 8 Trainium2 NeuronCores.

Math: with b0 = 0, softmax(b0, axis=1) is exactly uniform (1/N), so
outputs[b,i,k] = squash_k((1/N) * sum_j inputs_hat[b,j,k]) independent of i.
The b-update keeps b constant along axis 1, so softmax stays exactly uniform
and all routing iterations return the same outputs. Hence:

    Wsum[m,k] = sum_j W[j,m,k]
    v[b,k]    = (1/N) * (inputs @ Wsum)[b,k]
    out[b,i,k] = squash_k(v)[b,k]          (broadcast over i)

Sharding strategy:
  L1 (m-sharded): core c reduces W[:, 32c:32c+32, :] over j -> Wsum rows.
     W is staged to the device in bf16 (the 2e-2 rel-err budget dwarfs
     bf16's ~2e-3; the problem registry's references are bf16-native).
  L2 (batch-sharded): core c computes s_c = squash((inputs_c @ Wsum)/N)
     [64, 256] — the complete mathematical content of its output shard,
     since the i axis is degenerate.
  Unshard (host): concat s_c over batch and materialize the replicated
     i axis to the full [512, 256, 256] float32 output.
"""

import numpy as np
import ml_dtypes

import concourse.bass as bass
import concourse.mybir as mybir
import concourse.tile as tile
from concourse.ap import AP
from concourse.bass_utils import run_bass_kernel_spmd

F32 = mybir.dt.float32
BF16 = mybir.dt.bfloat16
NP_BF16 = ml_dtypes.bfloat16

B, N = 512, 256
NCORES = 8
BPC = B // NCORES  # 64 batch rows per core (L2)
MPC = N // NCORES  # 32 m rows per core (L1)
EPS = 1e-7

_CACHE = {}


def _fix_multiwait(nc, maxw=1):
    """This walrus build rejects instructions carrying more than one sync
    wait ("Too many sync wait commands"). Hoist extra waits into standalone
    single-wait EventSemaphore instructions on the same engine, placed
    immediately before the offender."""
    ctr = 0
    for fn in nc.m.functions:
        for bb in fn.blocks:
            out = []
            for ins in bb.instructions:
                si = ins.sync_info
                if si is not None and len(si.on_wait) > maxw:
                    waits = list(si.on_wait)
                    for w in waits[:-maxw]:
                        ctr += 1
                        ev = mybir.InstEventSemaphore(
                            name=f"mwsplit-{ctr}",
                            engine=ins.engine,
                            ins=[],
                            outs=[],
                            sync_info=mybir.SyncInfo(on_wait=[w], on_update=[]),
                        )
                        nc.register_instruction(ev, overwrite=True)
                        out.append(ev)
                    si.on_wait = waits[-maxw:]
                    ins.sync_info = si
                out.append(ins)
            bb.instructions[:] = out
    return nc

# Exec times (ns) of the last traced run, for test harnesses.
LAST_EXEC_NS = {"k1": None, "k2": None}


def _build_k1():
    """Reduce the per-core W slice over j (bf16 input, f32 accumulation).

    Input  w_in [128 (j%128), 16384 (jhalf*8192 + m_local*256 + k)] bf16
           (host packs the two j-halves of W[:, mslice, :] side by side)
    Output wsum_part [1, 8192] f32  (= Wsum[mslice, :] flat)

    Per chunk: ONE DMA covering both j-halves (sync queue), DVE adds the
    halves (j 256->128), PE ones-matmuls reduce the 128 partitions into
    PSUM f32, and copies (mostly scalar) drain PSUM into the f32 acc row.
    """
    nc = bass.Bass()
    FREE = MPC * N    # 8192
    MMF = 512         # moving free dim per matmul (one PSUM bank)

    w = nc.dram_tensor("w_in", [128, 2 * FREE], BF16, kind="ExternalInput")
    wsum = nc.dram_tensor("wsum_part", [1, FREE], F32, kind="ExternalOutput")

    # ~1 MB per chunk (both halves) keeps DMA efficiency; the small last
    # chunks shorten the serial tail after the final load.
    CHUNKS = [2048, 2048, 2048, 1024, 512, 512]
    assert sum(CHUNKS) == FREE

    with tile.TileContext(nc) as tc:
        with (
            tc.tile_pool(name="singles", bufs=1) as singles,
            tc.tile_pool(name="psum", bufs=8, space="PSUM") as psum_pool,
        ):
            ones = singles.tile([128, 1], BF16)
            nc.vector.memset(ones[:], 1.0)
            acc = singles.tile([1, FREE], F32)

            # Issue ALL chunk loads first (program order = engine queue
            # order) so no compute op can delay a load issue. The j<128
            # half loads on the sync HWDGE queue; the j>=128 half then
            # accumulates onto the same tile via SWDGE (CCE adds in the
            # DMA datapath), so DVE does no adds at all.
            tiles = []
            off = 0
            for ci, chunk in enumerate(CHUNKS):
                ta = singles.tile([128, chunk], BF16, tag=f"t{ci}")
                nc.sync.dma_start(out=ta[:], in_=w[:, off:off + chunk])
                nc.gpsimd.dma_start(
                    out=ta[:], in_=w[:, FREE + off:FREE + off + chunk],
                    accum_op=mybir.AluOpType.add,
                )
                tiles.append((off, chunk, ta))
                off += chunk

            gctr = 0
            for ci, (off, chunk, ts) in enumerate(tiles):
                for g in range(chunk // MMF):
                    ps = psum_pool.tile([1, MMF], F32)
                    gs = slice(g * MMF, (g + 1) * MMF)
                    nc.tensor.matmul(
                        ps[:], lhsT=ones[:], rhs=ts[:, gs],
                        start=True, stop=True,
                    )
                    osl = slice(off + g * MMF, off + (g + 1) * MMF)
                    # scalar takes most drains; vector gets every 3rd so
                    # the add pipeline keeps flowing.
                    gctr += 1
                    if gctr % 3 == 0:
                        nc.vector.tensor_copy(out=acc[0:1, osl], in_=ps[:])
                    else:
                        nc.scalar.copy(out=acc[0:1, osl], in_=ps[:])

            nc.sync.dma_start(out=wsum[:], in_=acc[:])
    return nc


def _build_k2():
    """Per-core: u = inputs_c @ Wsum, s = squash(u/N); write s [64, 256].

    Inputs  xt   [256 (m), 64 (b)]  bf16 (= inputs_c.T)
            wsum [256 (m), 256 (k)] bf16
    Output  s_out [64, 256] f32 = squash((inputs_c @ Wsum)/N)
    """
    nc = bass.Bass()
    # xw packs inputs_c.T and Wsum column-wise: [256 (m), 64 + 256]
    xw = nc.dram_tensor("xw", [N, BPC + N], BF16, kind="ExternalInput")
    s_out = nc.dram_tensor("s_out", [BPC, N], F32, kind="ExternalOutput")

    with tile.TileContext(nc) as tc:
        with (
            tc.tile_pool(name="sb", bufs=1) as sb,
            tc.tile_pool(name="psum", bufs=1, space="PSUM") as psum_pool,
        ):
            # Contraction dim m on partitions, split into two 128-halves.
            h0 = sb.tile([128, BPC + N], BF16)
            nc.sync.dma_start(out=h0[:], in_=xw[0:128, :])
            h1 = sb.tile([128, BPC + N], BF16)
            nc.scalar.dma_start(out=h1[:], in_=xw[128:256, :])

            # u[b, k] = sum_m inputs_c[b, m] * Wsum[m, k]
            u = psum_pool.tile([BPC, N], F32)
            nc.tensor.matmul(u[:], lhsT=h0[:, 0:BPC], rhs=h0[:, BPC:],
                             start=True, stop=False)
            nc.tensor.matmul(u[:], lhsT=h1[:, 0:BPC], rhs=h1[:, BPC:],
                             start=False, stop=True)

            # squash: v = u/N; s2 = sum_k v^2; s = v * s2/(1+s2)/sqrt(s2+eps)
            #       = u * factor,  factor = s2/(1+s2)/sqrt(s2+eps)/N
            sq = sb.tile([BPC, N], F32)
            s2 = sb.tile([BPC, 1], F32)
            nc.scalar.activation(
                out=sq[:], in_=u[:], func=mybir.ActivationFunctionType.Square,
                scale=1.0 / N, accum_out=s2[:],
            )
            eps_t = sb.tile([BPC, 1], F32)
            nc.vector.memset(eps_t[:], EPS)
            # parallel halves: scalar does r = sqrt(s2+eps) while vector
            # does q = (s2/N) / (1+s2); then s = u * q * (1/r) in one op.
            r = sb.tile([BPC, 1], F32)
            nc.scalar.activation(
                out=r[:], in_=s2[:], func=mybir.ActivationFunctionType.Sqrt,
                bias=eps_t[:],
            )
            r2 = sb.tile([BPC, 1], F32)
            nc.vector.reciprocal(r2[:], r[:])
            den1 = sb.tile([BPC, 1], F32)
            nc.vector.tensor_scalar(
                den1[:], s2[:], 1.0, None, mybir.AluOpType.add
            )
            p2 = sb.tile([BPC, 1], F32)
            nc.vector.reciprocal(p2[:], den1[:])
            q = sb.tile([BPC, 1], F32)
            nc.vector.scalar_tensor_tensor(
                q[:], s2[:], 1.0 / N, p2[:],
                op0=mybir.AluOpType.mult, op1=mybir.AluOpType.mult,
            )

            s_row = sb.tile([BPC, N], F32)
            nc.vector.tensor_scalar(
                s_row[:], u[:], q[:], r2[:],
                mybir.AluOpType.mult, mybir.AluOpType.mult,
            )
            nc.sync.dma_start(out=s_out[:], in_=s_row[:])
    return nc


def _run(nc, in_maps, core_ids, trace):
    if trace:
        try:
            return run_bass_kernel_spmd(nc, in_maps, core_ids, trace=True)
        except Exception as e:  # noqa: BLE001
            print(f"kernel: trace run failed ({e}); rerunning without trace")
    return run_bass_kernel_spmd(nc, in_maps, core_ids, trace=False)


def _get(name):
    if name not in _CACHE:
        _CACHE[name] = _fix_multiwait(_build_k1() if name == "k1" else _build_k2())
    return _CACHE[name]


def kernel(inputs: np.ndarray, W: np.ndarray, trace: bool = False) -> np.ndarray:
    inputs = np.ascontiguousarray(inputs, dtype=np.float32)
    W = np.ascontiguousarray(W, dtype=np.float32)
    core_ids = list(range(NCORES))

    # ---- L1: Wsum rows, m-sharded, bf16 ----
    k1 = _get("k1")
    w_bf = W.astype(NP_BF16)  # host-side staging cast
    # Pack the two j-halves side by side: [128, 2*8192]
    in_maps1 = [
        {
            "w_in": np.ascontiguousarray(
                w_bf[:, c * MPC:(c + 1) * MPC, :]
                .reshape(2, 128, MPC * N)
                .transpose(1, 0, 2)
                .reshape(128, 2 * MPC * N)
            )
        }
        for c in core_ids
    ]
    res1 = _run(k1, in_maps1, core_ids, trace)
    LAST_EXEC_NS["k1"] = res1.exec_time_ns
    wsum = np.concatenate(
        [res1.results[c]["wsum_part"].reshape(MPC, N) for c in core_ids], axis=0
    )  # [256, 256] f32

    # ---- L2: matmul + squash, batch-sharded ----
    k2 = _get("k2")
    xt_full = np.ascontiguousarray(inputs.T).astype(NP_BF16)  # [256, 512]
    wsum_bf = wsum.astype(NP_BF16)
    in_maps2 = [
        {
            "xw": np.ascontiguousarray(np.concatenate(
                [xt_full[:, c * BPC:(c + 1) * BPC], wsum_bf], axis=1
            )),
        }
        for c in core_ids
    ]
    res2 = _run(k2, in_maps2, core_ids, trace)
    LAST_EXEC_NS["k2"] = res2.exec_time_ns

    # ---- unshard: concat batch shards, materialize the replicated i axis ----
    s = np.concatenate(
        [res2.results[c]["s_out"] for c in core_ids], axis=0
    )  # [512, 256] f32
    out = np.ascontiguousarray(
        np.broadcast_to(s[:, None, :], (B, N, N))
    )
    return out
